# revision 17
# baseline (speedup 1.0000x reference)
"""Trainium2 Bass kernel for nn_AutoregressivePrior.

Computes a K-step tiny-LSTM autoregressive prior (HID=256), projects each
step's hidden state to (loc, scale) rows of width 64, and materializes the
batch-broadcast output [K*batch_size, 64] for both loc and scale.

Strategy (8 NeuronCores, SPMD):
  - The LSTM recurrence + projections are tiny and replicated on every core.
  - The broadcast/repeat over batch_size (the memory-bound part) is sharded:
    each core writes its own batch_size/8 = 4096-row slice of every output
    row k, as one contiguous 2 MB DMA per k.

Design notes:
  - The LSTM state lives in COLUMN layout [128 partitions, 2]: gate
    pre-activations are computed as gates^T [128, 8] with the weight chunk
    as the stationary matmul operand, so every elementwise/activation op
    runs on 128 lanes instead of 1, and the hidden state needs no
    transpose between steps.
  - All matmuls use float32r (full-rate fp32 on the PE; plain fp32 streams
    at 1/4 rate). End-to-end error vs the fp32 reference is ~5e-5.
  - After step 0, x and h are both h_new, so gates = (W_ih + W_hh) @ h + b.
  - Gate rows are permuted to (i, f, o, g) so one sigmoid covers gate
    columns 0:6 and one tanh covers 6:8; the gate bias is added by a
    single [128, 8] DVE add against a host-prepacked bias-column tile.
  - Constants arrive via three packed HWDGE DMA transfers (small proj
    consts first, then the two weight matrices), so every consumer
    instruction needs at most one semaphore wait.
"""

import numpy as np

import concourse.bacc as bacc
import concourse.mybir as mybir
from concourse.tile import TileContext
from concourse.bass_utils import run_bass_kernel_spmd

F32 = mybir.dt.float32
F32R = mybir.dt.float32r

HID = 256
K = 7
BATCH = 32768
NCORES = 8
BS = BATCH // NCORES  # 4096 batch rows per core
P = 128               # partitions
RPP = BS // P         # 32 batch rows per partition
ZM = 64               # zm_size

# --- packed const layouts (column offsets) ---
# megaA (f32r): projection/broadcast consts + gate bias columns
MA_WL = 0                  # wlst chunks (c p n): cols [0, 256)
MA_ZC = 256                # zm1 column form, duplicated pairs: [256, 260)
MA_BGC = 260               # gate bias columns, duplicated [128, 16]: [260, 276)
MA_BLS = 276               # partition-0 row: biasls [276, 404)
MA_ONE = 404               # partition-0 row: ones  [404, 532)
MA_W = 532
# megaB1 (f32r): step-1 weights + step-1 input column
MB1_W0 = 0                 # w0t chunks: [0, 2048)
MB1_ZC = 2048              # zm1 column form, duplicated pairs: [2048, 2052)
MB1_W = 2052
# megaB2 (f32r): steady-state weights
MB2_WS = 0                 # wst chunks: [0, 2048)
MB2_W = 2048

_NC_CACHE = {}


def build_nc():
    nc = bacc.Bacc("TRN2", target_bir_lowering=False, debug=False)

    megaA_d = nc.declare_dram_parameter("megaA", [P, MA_W], F32R, isOutput=False)
    megaB1_d = nc.declare_dram_parameter("megaB1", [P, MB1_W], F32R, isOutput=False)
    megaB2_d = nc.declare_dram_parameter("megaB2", [P, MB2_W], F32R, isOutput=False)
    out_d = nc.declare_dram_parameter("out", [K, 2, BS, ZM], F32, isOutput=True)

    with TileContext(nc) as tc:
        with (
            tc.tile_pool(name="const", bufs=1) as cpool,
            tc.tile_pool(name="state", bufs=3) as spool,
            tc.tile_pool(name="hcol", bufs=3) as hpool,
            tc.tile_pool(name="row", bufs=3) as rpool,
            tc.tile_pool(name="wide", bufs=3) as wpool,
            tc.tile_pool(name="pgates", bufs=2, space="PSUM") as pg_pool,
            tc.tile_pool(name="pls", bufs=2, space="PSUM") as pls_pool,
            tc.tile_pool(name="pbcast", bufs=2, space="PSUM") as pb_pool,
            tc.tile_pool(name="pwarm", bufs=1, space="PSUM") as pw_pool,
        ):
            ma = cpool.tile([P, MA_W], F32R)
            nc.sync.dma_start(out=ma[:], in_=megaA_d[:])
            mb1 = cpool.tile([P, MB1_W], F32R)
            nc.sync.dma_start(out=mb1[:], in_=megaB1_d[:])
            mb2 = cpool.tile([P, MB2_W], F32R)
            nc.sync.dma_start(out=mb2[:], in_=megaB2_d[:])

            wlst_sb = ma[:, MA_WL : MA_WL + 256]
            zm1c_a = ma[:, MA_ZC : MA_ZC + 4]
            bgc_f32 = ma[:, MA_BGC : MA_BGC + 16].bitcast(F32)
            biasls_r = ma[0:1, MA_BLS : MA_BLS + 128]
            ones_row = ma[0:1, MA_ONE : MA_ONE + 128]
            one_1x1 = ma[0:1, MA_ONE : MA_ONE + 1]

            def emit_row(k, xcr):
                """Project p_z[k] (f32r column form xcr [128, 2]) to loc|scale
                and write this core's batch-broadcast slice of output row k."""
                pls = pls_pool.tile([1, 2 * ZM], F32)
                nc.tensor.matmul(
                    pls[:], lhsT=xcr[:, 0:1], rhs=wlst_sb[:, 0:128],
                    start=True, stop=False,
                )
                nc.tensor.matmul(
                    pls[:], lhsT=xcr[:, 2:3], rhs=wlst_sb[:, 128:256],
                    start=False, stop=False,
                )
                nc.tensor.matmul(
                    pls[:], lhsT=one_1x1, rhs=biasls_r, start=False, stop=True,
                )
                lsrow = rpool.tile([1, 2 * ZM], F32R)
                nc.vector.tensor_copy(out=lsrow[:], in_=pls[:])
                # broadcast across all 128 partitions: outer product with ones
                pb = pb_pool.tile([P, 2 * ZM], F32)
                nc.tensor.matmul(
                    pb[:], lhsT=ones_row, rhs=lsrow[0:1, :], start=True, stop=True,
                )
                bc = rpool.tile([P, 2 * ZM], F32)
                nc.vector.tensor_copy(out=bc[:], in_=pb[:])
                # widen + store loc and scale separately: each half's 1 MB DMA
                # can start while the other half's widen is still running
                for t in (0, 1):
                    wide = wpool.tile([P, RPP * ZM], F32, tag=f"wide{t}")
                    nc.vector.tensor_copy(
                        out=wide[:].rearrange("p (r j) -> p r j", r=RPP),
                        in_=bc[:, t * ZM : (t + 1) * ZM][:, None, :].broadcast_to(
                            (P, RPP, ZM)
                        ),
                    )
                    nc.sync.dma_start(
                        out=out_d[k, t].rearrange("(p r) j -> p r j", p=P),
                        in_=wide[:].rearrange("p (r j) -> p r j", r=RPP),
                    )

            def emit_step(t, xcr_prev, st_prev):
                """One LSTM cell step, duplicated-pair column layout.

                Gate m's 128 pre-activations land in BOTH columns 2m and
                2m+1 of pgc (fp32r matmuls need a moving free dim >= 2, so
                each is fed the same input column twice). All elementwise
                ops run on the duplicated tiles, which makes h come out
                pre-duplicated as the next step's rhs pairs.

                st tiles hold [tanh(g) (0:4) | c (4:8)]: this step's tanh(g)
                is written into st_prev's first half, so one fused multiply
                computes both i*tanh(g) and f*c.
                Returns (st_next, h16); h16 is [128, 4] = [h0 h0 h1 h1]."""
                wsb = mb1 if t == 1 else mb2
                wofs = MB1_W0 if t == 1 else MB2_WS
                pgc = pg_pool.tile([P, 16], F32)
                for m in range(8):
                    for c in (0, 1):
                        nc.tensor.matmul(
                            pgc[:, 2 * m : 2 * m + 2],
                            lhsT=wsb[:, wofs + c * 1024 + m * 128 : wofs + c * 1024 + (m + 1) * 128],
                            rhs=xcr_prev[:, 2 * c : 2 * c + 2],
                            start=(c == 0), stop=(c == 1),
                        )
                gb = spool.tile([P, 16], F32)
                nc.vector.tensor_add(out=gb[:], in0=pgc[:], in1=bgc_f32)
                # duplicated gate columns (i, f, o, g) = (0:4, 4:8, 8:12, 12:16)
                ga = spool.tile([P, 12], F32)
                nc.scalar.activation(
                    out=ga[:], in_=gb[:, 0:12],
                    func=mybir.ActivationFunctionType.Sigmoid,
                )
                nc.scalar.activation(
                    out=st_prev[:, 0:4], in_=gb[:, 12:16],
                    func=mybir.ActivationFunctionType.Tanh,
                )
                st_next = spool.tile([P, 8], F32, tag="st")
                if t == 1:
                    # c0 = 0: c1 = i*tanh(g) directly into st_next's c half
                    nc.vector.tensor_mul(
                        out=st_next[:, 4:8], in0=ga[:, 0:4], in1=st_prev[:, 0:4]
                    )
                else:
                    t12 = spool.tile([P, 8], F32)
                    nc.vector.tensor_mul(out=t12[:], in0=ga[:, 0:8], in1=st_prev[:])
                    nc.vector.tensor_add(
                        out=st_next[:, 4:8], in0=t12[:, 0:4], in1=t12[:, 4:8]
                    )
                tc_ = spool.tile([P, 4], F32)
                nc.scalar.activation(
                    out=tc_[:], in_=st_next[:, 4:8],
                    func=mybir.ActivationFunctionType.Tanh,
                )
                h16 = hpool.tile([P, 4], F32R)
                nc.vector.tensor_mul(out=h16[:], in0=ga[:, 8:12], in1=tc_[:])
                # keepalive: a tiny dep-free-output matmul gated on this step's
                # tanh(c) keeps the PE HAM window from re-throttling during the
                # elementwise chain (the idle gap would otherwise cross ~3.4us)
                pw = pw_pool.tile([2, 4], F32)
                nc.tensor.matmul(
                    pw[:], lhsT=tc_[:, 0:2], rhs=tc_[:, 0:4],
                    start=True, stop=True, skip_group_check=True,
                )
                return st_next, h16

            emit_row(0, zm1c_a)
            xcr = mb1[:, MB1_ZC : MB1_ZC + 4]
            st = spool.tile([P, 8], F32, tag="st")
            for t in range(1, K):
                st, xcr = emit_step(t, xcr, st)
                emit_row(t, xcr)

    nc.compile()
    return nc


def _get_nc():
    if "nc" not in _NC_CACHE:
        _NC_CACHE["nc"] = build_nc()
    return _NC_CACHE["nc"]


def prepare_inputs(**inputs):
    """Host-side prep: pure numpy reshuffling of the full inputs into the
    per-core input map (identical on every core)."""
    f = lambda k: np.asarray(inputs[k], dtype=np.float32)
    zm_1, W_ih, W_hh = f("zm_1"), f("W_ih"), f("W_hh")
    b_ih, b_hh = f("b_ih"), f("b_hh")
    W_loc, b_loc, W_scale, b_scale = f("W_loc"), f("b_loc"), f("W_scale"), f("b_scale")
    assert int(inputs["K"]) == K and int(inputs["batch_size"]) == BATCH

    def cpn(wt):
        # [256, N] -> chunked [128, 2*N]: chunk c (rows c*128..) at cols [c*N, (c+1)*N)
        n = wt.shape[1]
        return wt.reshape(2, P, n).transpose(1, 0, 2).reshape(P, 2 * n)

    # reorder gates (i, f, g, o) -> (i, f, o, g)
    perm = np.r_[0:256, 256:512, 768:1024, 512:768]
    w0t = W_ih[perm].T                 # [256, 1024]
    wst = (W_ih + W_hh)[perm].T        # [256, 1024]
    biasg = (b_ih + b_hh)[perm]        # [1024]
    wlst = np.concatenate([W_loc.T, W_scale.T], axis=1)  # [256, 128]
    biasls = np.concatenate([b_loc, b_scale])            # [128]
    zm1c = zm_1.reshape(2, P).T                          # [128, 2]

    zm1c_dup = np.repeat(zm1c, 2, axis=1)                # [128, 4]

    ma = np.zeros((P, MA_W), np.float32)
    ma[:, MA_WL : MA_WL + 256] = cpn(wlst)
    ma[:, MA_ZC : MA_ZC + 4] = zm1c_dup
    ma[:, MA_BGC : MA_BGC + 16] = np.repeat(biasg.reshape(8, P).T, 2, axis=1)
    ma[0, MA_BLS : MA_BLS + 128] = biasls
    ma[0, MA_ONE : MA_ONE + 128] = 1.0

    mb1 = np.zeros((P, MB1_W), np.float32)
    mb1[:, MB1_W0 : MB1_W0 + 2048] = cpn(w0t)
    mb1[:, MB1_ZC : MB1_ZC + 4] = zm1c_dup

    mb2 = np.zeros((P, MB2_W), np.float32)
    mb2[:, MB2_WS : MB2_WS + 2048] = cpn(wst)

    return {"megaA": ma, "megaB1": mb1, "megaB2": mb2}


def execute(in_map, **kwargs):
    nc = _get_nc()
    return run_bass_kernel_spmd(
        nc, [dict(in_map) for _ in range(NCORES)], core_ids=list(range(NCORES)), **kwargs
    )


def assemble_output(results):
    loc = np.empty((K, BATCH, ZM), np.float32)
    scale = np.empty((K, BATCH, ZM), np.float32)
    for c in range(NCORES):
        o = results[c]["out"]  # [K, 2, BS, ZM]
        loc[:, c * BS : (c + 1) * BS] = o[:, 0]
        scale[:, c * BS : (c + 1) * BS] = o[:, 1]
    return loc.reshape(-1, ZM), scale.reshape(-1, ZM)


def kernel(**inputs):
    in_map = prepare_inputs(**inputs)
    res = execute(in_map)
    return assemble_output(res.results)


# revision 18
# speedup vs baseline: 1.0198x; 1.0198x over previous
"""Trainium2 Bass kernel for nn_AutoregressivePrior.

Computes a K-step tiny-LSTM autoregressive prior (HID=256), projects each
step's hidden state to (loc, scale) rows of width 64, and materializes the
batch-broadcast output [K*batch_size, 64] for both loc and scale.

Strategy (8 NeuronCores, SPMD):
  - The LSTM recurrence + projections are tiny and replicated on every core.
  - The broadcast/repeat over batch_size (the memory-bound part) is sharded:
    each core writes its own batch_size/8 = 4096-row slice of every output
    row k, as one contiguous 2 MB DMA per k.

Design notes:
  - The LSTM state lives in COLUMN layout [128 partitions, 2]: gate
    pre-activations are computed as gates^T [128, 8] with the weight chunk
    as the stationary matmul operand, so every elementwise/activation op
    runs on 128 lanes instead of 1, and the hidden state needs no
    transpose between steps.
  - All matmuls use float32r (full-rate fp32 on the PE; plain fp32 streams
    at 1/4 rate). End-to-end error vs the fp32 reference is ~5e-5.
  - After step 0, x and h are both h_new, so gates = (W_ih + W_hh) @ h + b.
  - Gate rows are permuted to (i, f, o, g) so one sigmoid covers gate
    columns 0:6 and one tanh covers 6:8; the gate bias is added by a
    single [128, 8] DVE add against a host-prepacked bias-column tile.
  - Constants arrive via three packed HWDGE DMA transfers (small proj
    consts first, then the two weight matrices), so every consumer
    instruction needs at most one semaphore wait.
"""

import numpy as np

import concourse.bacc as bacc
import concourse.mybir as mybir
from concourse.tile import TileContext
from concourse.bass_utils import run_bass_kernel_spmd

F32 = mybir.dt.float32
F32R = mybir.dt.float32r

HID = 256
K = 7
BATCH = 32768
NCORES = 8
BS = BATCH // NCORES  # 4096 batch rows per core
P = 128               # partitions
RPP = BS // P         # 32 batch rows per partition
ZM = 64               # zm_size

# --- packed const layouts (column offsets) ---
# megaA (f32r): projection/broadcast consts + gate bias columns
MA_WL = 0                  # wlst chunks (c p n): cols [0, 256)
MA_ZC = 256                # zm1 column form, duplicated pairs: [256, 260)
MA_BGC = 260               # gate bias columns, duplicated [128, 16]: [260, 276)
MA_BLS = 276               # partition-0 row: biasls [276, 404)
MA_ONE = 404               # partition-0 row: ones  [404, 532)
MA_W = 532
# megaB1 (f32r): step-1 weights + step-1 input column
MB1_W0 = 0                 # w0t chunks: [0, 2048)
MB1_ZC = 2048              # zm1 column form, duplicated pairs: [2048, 2052)
MB1_W = 2052
# megaB2 (f32r): steady-state weights
MB2_WS = 0                 # wst chunks: [0, 2048)
MB2_W = 2048

_NC_CACHE = {}


def build_nc():
    nc = bacc.Bacc("TRN2", target_bir_lowering=False, debug=False)

    megaA_d = nc.declare_dram_parameter("megaA", [P, MA_W], F32R, isOutput=False)
    megaB1_d = nc.declare_dram_parameter("megaB1", [P, MB1_W], F32R, isOutput=False)
    megaB2_d = nc.declare_dram_parameter("megaB2", [P, MB2_W], F32R, isOutput=False)
    out_d = nc.declare_dram_parameter("out", [K, 2, BS, ZM], F32, isOutput=True)

    with TileContext(nc) as tc:
        with (
            tc.tile_pool(name="const", bufs=1) as cpool,
            tc.tile_pool(name="state", bufs=3) as spool,
            tc.tile_pool(name="hcol", bufs=3) as hpool,
            tc.tile_pool(name="row", bufs=3) as rpool,
            tc.tile_pool(name="wide", bufs=3) as wpool,
            tc.tile_pool(name="pgates", bufs=2, space="PSUM") as pg_pool,
            tc.tile_pool(name="pgo", bufs=2, space="PSUM") as pgb_pool,
            tc.tile_pool(name="pls", bufs=2, space="PSUM") as pls_pool,
            tc.tile_pool(name="pbcast", bufs=2, space="PSUM") as pb_pool,
        ):
            ma = cpool.tile([P, MA_W], F32R)
            nc.sync.dma_start(out=ma[:], in_=megaA_d[:])
            mb1 = cpool.tile([P, MB1_W], F32R)
            nc.sync.dma_start(out=mb1[:], in_=megaB1_d[:])
            mb2 = cpool.tile([P, MB2_W], F32R)
            nc.sync.dma_start(out=mb2[:], in_=megaB2_d[:])

            wlst_sb = ma[:, MA_WL : MA_WL + 256]
            zm1c_a = ma[:, MA_ZC : MA_ZC + 4]
            bgcA_f32 = ma[:, MA_BGC : MA_BGC + 12].bitcast(F32)
            bgcB_f32 = ma[:, MA_BGC + 12 : MA_BGC + 16].bitcast(F32)
            biasls_r = ma[0:1, MA_BLS : MA_BLS + 128]
            ones_row = ma[0:1, MA_ONE : MA_ONE + 128]
            one_1x1 = ma[0:1, MA_ONE : MA_ONE + 1]

            def emit_row(k, xcr):
                """Project p_z[k] (f32r column form xcr [128, 2]) to loc|scale
                and write this core's batch-broadcast slice of output row k."""
                pls = pls_pool.tile([1, 2 * ZM], F32)
                nc.tensor.matmul(
                    pls[:], lhsT=xcr[:, 0:1], rhs=wlst_sb[:, 0:128],
                    start=True, stop=False,
                )
                nc.tensor.matmul(
                    pls[:], lhsT=xcr[:, 2:3], rhs=wlst_sb[:, 128:256],
                    start=False, stop=False,
                )
                nc.tensor.matmul(
                    pls[:], lhsT=one_1x1, rhs=biasls_r, start=False, stop=True,
                )
                lsrow = rpool.tile([1, 2 * ZM], F32R)
                nc.vector.tensor_copy(out=lsrow[:], in_=pls[:])
                # broadcast across all 128 partitions: outer product with ones
                pb = pb_pool.tile([P, 2 * ZM], F32)
                nc.tensor.matmul(
                    pb[:], lhsT=ones_row, rhs=lsrow[0:1, :], start=True, stop=True,
                )
                bc = rpool.tile([P, 2 * ZM], F32)
                nc.vector.tensor_copy(out=bc[:], in_=pb[:])
                # widen + store loc and scale separately: each half's 1 MB DMA
                # can start while the other half's widen is still running
                for t in (0, 1):
                    wide = wpool.tile([P, RPP * ZM], F32, tag=f"wide{t}")
                    nc.vector.tensor_copy(
                        out=wide[:].rearrange("p (r j) -> p r j", r=RPP),
                        in_=bc[:, t * ZM : (t + 1) * ZM][:, None, :].broadcast_to(
                            (P, RPP, ZM)
                        ),
                    )
                    nc.sync.dma_start(
                        out=out_d[k, t].rearrange("(p r) j -> p r j", p=P),
                        in_=wide[:].rearrange("p (r j) -> p r j", r=RPP),
                    )

            def emit_step(t, xcr_prev, st_prev):
                """One LSTM cell step, duplicated-pair column layout.

                Gate m's 128 pre-activations land in BOTH columns 2m and
                2m+1 (fp32r matmuls need a moving free dim >= 2, so each is
                fed the same input column twice). All elementwise ops run on
                the duplicated tiles, which makes h come out pre-duplicated
                as the next step's rhs pairs.

                Native gate order (i, f, g, o): the i/f/g chunks (pgA) are
                issued first and start the serial chain; the o chunks (pgB)
                are only needed for the final h multiply, so their matmuls
                and sigmoid run concurrently with the chain.

                st tiles hold [tanh(g) (0:4) | c (4:8)]: this step's tanh(g)
                is written into st_prev's first half, so one fused multiply
                computes both i*tanh(g) and f*c.
                Returns (st_next, h16); h16 is [128, 4] = [h0 h0 h1 h1]."""
                wsb = mb1 if t == 1 else mb2
                wofs = MB1_W0 if t == 1 else MB2_WS
                pgA = pg_pool.tile([P, 12], F32)   # i (0:4), f (4:8), g (8:12)
                pgB = pgb_pool.tile([P, 4], F32)   # o
                for m in range(8):
                    dst = pgA[:, 2 * m : 2 * m + 2] if m < 6 else pgB[:, 2 * (m - 6) : 2 * (m - 6) + 2]
                    for c in (0, 1):
                        nc.tensor.matmul(
                            dst,
                            lhsT=wsb[:, wofs + c * 1024 + m * 128 : wofs + c * 1024 + (m + 1) * 128],
                            rhs=xcr_prev[:, 2 * c : 2 * c + 2],
                            start=(c == 0), stop=(c == 1),
                        )
                gbA = spool.tile([P, 12], F32)
                nc.vector.tensor_add(out=gbA[:], in0=pgA[:], in1=bgcA_f32)
                ga = spool.tile([P, 8], F32)       # sigmoid(i | f)
                nc.scalar.activation(
                    out=ga[:], in_=gbA[:, 0:8],
                    func=mybir.ActivationFunctionType.Sigmoid,
                )
                nc.scalar.activation(
                    out=st_prev[:, 0:4], in_=gbA[:, 8:12],
                    func=mybir.ActivationFunctionType.Tanh,
                )
                # o-gate path, concurrent with the c chain
                gbB = spool.tile([P, 4], F32)
                nc.vector.tensor_add(out=gbB[:], in0=pgB[:], in1=bgcB_f32)
                so = spool.tile([P, 4], F32)
                nc.scalar.activation(
                    out=so[:], in_=gbB[:],
                    func=mybir.ActivationFunctionType.Sigmoid,
                )
                st_next = spool.tile([P, 8], F32, tag="st")
                if t == 1:
                    # c0 = 0: c1 = i*tanh(g) directly into st_next's c half
                    nc.vector.tensor_mul(
                        out=st_next[:, 4:8], in0=ga[:, 0:4], in1=st_prev[:, 0:4]
                    )
                else:
                    t12 = spool.tile([P, 8], F32)
                    nc.vector.tensor_mul(out=t12[:], in0=ga[:], in1=st_prev[:])
                    nc.vector.tensor_add(
                        out=st_next[:, 4:8], in0=t12[:, 0:4], in1=t12[:, 4:8]
                    )
                tc_ = spool.tile([P, 4], F32)
                nc.scalar.activation(
                    out=tc_[:], in_=st_next[:, 4:8],
                    func=mybir.ActivationFunctionType.Tanh,
                )
                h16 = hpool.tile([P, 4], F32R)
                nc.vector.tensor_mul(out=h16[:], in0=so[:], in1=tc_[:])
                return st_next, h16

            emit_row(0, zm1c_a)
            xcr = mb1[:, MB1_ZC : MB1_ZC + 4]
            st = spool.tile([P, 8], F32, tag="st")
            for t in range(1, K):
                st, xcr = emit_step(t, xcr, st)
                emit_row(t, xcr)

    nc.compile()
    return nc


def _get_nc():
    if "nc" not in _NC_CACHE:
        _NC_CACHE["nc"] = build_nc()
    return _NC_CACHE["nc"]


def prepare_inputs(**inputs):
    """Host-side prep: pure numpy reshuffling of the full inputs into the
    per-core input map (identical on every core)."""
    f = lambda k: np.asarray(inputs[k], dtype=np.float32)
    zm_1, W_ih, W_hh = f("zm_1"), f("W_ih"), f("W_hh")
    b_ih, b_hh = f("b_ih"), f("b_hh")
    W_loc, b_loc, W_scale, b_scale = f("W_loc"), f("b_loc"), f("W_scale"), f("b_scale")
    assert int(inputs["K"]) == K and int(inputs["batch_size"]) == BATCH

    def cpn(wt):
        # [256, N] -> chunked [128, 2*N]: chunk c (rows c*128..) at cols [c*N, (c+1)*N)
        n = wt.shape[1]
        return wt.reshape(2, P, n).transpose(1, 0, 2).reshape(P, 2 * n)

    # native gate order (i, f, g, o): i/f/g feed the serial chain, o overlaps it
    w0t = W_ih.T                       # [256, 1024]
    wst = (W_ih + W_hh).T              # [256, 1024]
    biasg = b_ih + b_hh                # [1024]
    wlst = np.concatenate([W_loc.T, W_scale.T], axis=1)  # [256, 128]
    biasls = np.concatenate([b_loc, b_scale])            # [128]
    zm1c = zm_1.reshape(2, P).T                          # [128, 2]

    zm1c_dup = np.repeat(zm1c, 2, axis=1)                # [128, 4]

    ma = np.zeros((P, MA_W), np.float32)
    ma[:, MA_WL : MA_WL + 256] = cpn(wlst)
    ma[:, MA_ZC : MA_ZC + 4] = zm1c_dup
    ma[:, MA_BGC : MA_BGC + 16] = np.repeat(biasg.reshape(8, P).T, 2, axis=1)
    ma[0, MA_BLS : MA_BLS + 128] = biasls
    ma[0, MA_ONE : MA_ONE + 128] = 1.0

    mb1 = np.zeros((P, MB1_W), np.float32)
    mb1[:, MB1_W0 : MB1_W0 + 2048] = cpn(w0t)
    mb1[:, MB1_ZC : MB1_ZC + 4] = zm1c_dup

    mb2 = np.zeros((P, MB2_W), np.float32)
    mb2[:, MB2_WS : MB2_WS + 2048] = cpn(wst)

    return {"megaA": ma, "megaB1": mb1, "megaB2": mb2}


def execute(in_map, **kwargs):
    nc = _get_nc()
    return run_bass_kernel_spmd(
        nc, [dict(in_map) for _ in range(NCORES)], core_ids=list(range(NCORES)), **kwargs
    )


def assemble_output(results):
    loc = np.empty((K, BATCH, ZM), np.float32)
    scale = np.empty((K, BATCH, ZM), np.float32)
    for c in range(NCORES):
        o = results[c]["out"]  # [K, 2, BS, ZM]
        loc[:, c * BS : (c + 1) * BS] = o[:, 0]
        scale[:, c * BS : (c + 1) * BS] = o[:, 1]
    return loc.reshape(-1, ZM), scale.reshape(-1, ZM)


def kernel(**inputs):
    in_map = prepare_inputs(**inputs)
    res = execute(in_map)
    return assemble_output(res.results)
